# revision 2
# baseline (speedup 1.0000x reference)
"""Trainium2 Bass kernel for nn_Attention_40261023433214 (retrieval_knn).

Computation (per image):
  q = conv1x1(feat_edit, wq, bq); k = conv1x1(feat_ori, wk, bk)
  qu = unfold(q, 16); ku = unfold(k, 16); ku normalized per patch
  energy_T[m, n] = qu[m] . ku_norm[n]   (q-norm skipped: positive per-m scale
                                         doesn't change argmax/argmin over n)
  am = argmax_n energy_T; an = argmin_n
  out = fold(unfold(x1)[am]) + gamma2 * fold(unfold(x2)[an])

Host/device split (the host<->device link runs ~40 MB/s up / ~20 MB/s down,
so transferred bytes dominate the wall clock):
  host:   1x1 conv folded into the upload step (3->1 channel compression, fp16)
  device: unfold, k-normalization, all [1024x256]x[256x1024] energy matmuls,
          top-8/bottom-8 selection per query
  host:   exact fp32 re-rank of the <=8 candidates for the ~12% of queries
          whose fp16 margin is below a sound threshold (2*delta bound), then
          the final index-select (gather+fold).
Only 34 MB of fp16 q/k maps go up and ~1.3 MB of candidates come back.
The conv streams out in 4 chunks (async device_put) so the CPU conv of chunk
j overlaps the network upload of chunk j-1.
"""
import sys
sys.path.insert(0, '/opt/trn_rl_repo')
import numpy as np

try:
    import torch as _torch
except Exception:          # pragma: no cover - torch is present in this image
    _torch = None

B, C, H, W = 32, 3, 512, 512
KP = 16                 # patch size
NB = H // KP            # 32 patch blocks per side
N = NB * NB             # 1024 patches
PD = KP * KP            # 256 positions per patch (single channel)
N_CORES = 8
IPC = B // N_CORES      # 4 images per core; image b = core*IPC + j lives in chunk j
TH = 0.02               # refine threshold; measured max |e_fp16 - e_fp32| ~ 1.1e-3

_CACHE = {}


def _build(with_x2: bool):
    import concourse.bass as bass
    import concourse.mybir as mybir
    from concourse.tile import TileContext
    from concourse.masks import make_identity

    F32 = mybir.dt.float32
    F16 = mybir.dt.float16
    U32 = mybir.dt.uint32
    AF = mybir.ActivationFunctionType
    ALU = mybir.AluOpType

    NO = 2 if with_x2 else 1

    nc = bass.Bass()
    # one parameter per local image so the host can pipeline conv with upload
    qk_ds = [nc.declare_dram_parameter(f"qk{j}", [1, 2, H, W], F16, isOutput=False)
             for j in range(IPC)]
    cv_d = nc.declare_dram_parameter("cv", [IPC, NO, N, 10], U32, isOutput=True)

    # internal DRAM scratch for the unfold round trip
    qpm_d = nc.dram_tensor("qpm", [IPC, N, PD], F16)       # q patch-major
    kpm_d = nc.dram_tensor("kpm", [IPC, N, PD], F16)

    # views
    qk_bigs = [t.rearrange("i t (p hs) w -> i t p (hs w)", hs=4)[0] for t in qk_ds]
    qpm_v = qpm_d.rearrange("i (hb wb) (r s) -> i hb wb r s", wb=NB, r=KP)
    kpm_v = kpm_d.rearrange("i (hb wb) (r s) -> i hb wb r s", wb=NB, r=KP)

    def dual(idx):
        return nc.sync if idx % 2 == 0 else nc.scalar

    with TileContext(nc) as tc:
        with (
            tc.tile_pool(name="cst", bufs=1) as cst,
            tc.tile_pool(name="feat", bufs=3) as featp,
            tc.tile_pool(name="pm", bufs=6) as pmp,
            tc.tile_pool(name="tiny", bufs=8) as tinyp,
            tc.tile_pool(name="qu", bufs=8) as qup,
            tc.tile_pool(name="esb", bufs=4) as esbp,
            tc.tile_pool(name="idx", bufs=4) as idxp,
            tc.tile_pool(name="pst", bufs=4, space="PSUM") as pstp,
            tc.tile_pool(name="pse", bufs=4, space="PSUM") as psep,
        ):
            idn = cst.tile([128, 128], F16, name="idn")
            make_identity(nc, idn[:])

            for b in range(IPC):
                # ---- load q/k natural [128, 2048] (h = 4p + hs) ----
                qbig = featp.tile([128, 2048], F16, name="qbig", tag="feat")
                nc.sync.dma_start(out=qbig[:], in_=qk_bigs[b][0])
                kbig = featp.tile([128, 2048], F16, name="kbig", tag="feat")
                nc.scalar.dma_start(out=kbig[:], in_=qk_bigs[b][1])

                # ---- unfold via d2d: SBUF[h,w] -> DRAM pm [N, 256] ----
                for src, dst_v in ((qbig, qpm_v), (kbig, kpm_v)):
                    v = src.rearrange("(hb pr) (hs wb s) -> hb pr hs wb s", pr=4, hs=4, s=KP)
                    for r in range(KP):
                        dual(r).dma_start(out=dst_v[b, :, :, r, :], in_=v[:, r // 4, r % 4])

                # ---- reload pm tiles, k-norm, transposes to [(r,s), n] ----
                qu_lo = qup.tile([128, N], F16, name="qulo", tag="qu")
                qu_hi = qup.tile([128, N], F16, name="quhi", tag="qu")
                ku_lo = qup.tile([128, N], F16, name="kulo", tag="qu")
                ku_hi = qup.tile([128, N], F16, name="kuhi", tag="qu")
                for t in range(8):
                    qt = pmp.tile([128, PD], F16, name="qt", tag="pm")
                    nc.sync.dma_start(out=qt[:], in_=qpm_d[b, 128 * t:128 * (t + 1), :])
                    kt = pmp.tile([128, PD], F16, name="kt", tag="pm")
                    nc.scalar.dma_start(out=kt[:], in_=kpm_d[b, 128 * t:128 * (t + 1), :])
                    # k norm (per patch = per partition here)
                    dump = pmp.tile([128, PD], F32, name="dump", tag="pmd")
                    ssq = tinyp.tile([128, 1], F32, name="ssq", tag="tiny")
                    nc.scalar.activation(dump[:], kt[:], AF.Square, accum_out=ssq[:])
                    nrm = tinyp.tile([128, 1], F32, name="nrm", tag="tiny")
                    nc.scalar.sqrt(nrm[:], ssq[:])
                    inv = tinyp.tile([128, 1], F32, name="inv", tag="tiny")
                    nc.vector.reciprocal(inv[:], nrm[:])
                    kn = pmp.tile([128, PD], F16, name="kn", tag="pm")
                    nc.vector.tensor_scalar(out=kn[:], in0=kt[:], scalar1=inv[:, 0:1],
                                            scalar2=None, op0=ALU.mult)
                    # transposes into qu/ku tiles ([rs, n] layout, fp16)
                    for half, (qdst, kdst) in enumerate(((qu_lo, ku_lo), (qu_hi, ku_hi))):
                        pt = pstp.tile([128, 128], F16, name="ptq", tag="pst", space="PSUM")
                        nc.tensor.transpose(pt[:], qt[:, 128 * half:128 * (half + 1)], idn[:])
                        nc.scalar.copy(qdst[:, 128 * t:128 * (t + 1)], pt[:])
                        pk = pstp.tile([128, 128], F16, name="ptk", tag="pst", space="PSUM")
                        nc.tensor.transpose(pk[:], kn[:, 128 * half:128 * (half + 1)], idn[:])
                        nc.scalar.copy(kdst[:, 128 * t:128 * (t + 1)], pk[:])

                # ---- energy + top-8 (and bottom-8) per m-chunk ----
                for mt in range(8):
                    esb = esbp.tile([128, N], F32, name="esb", tag="esb")
                    for nf in range(2):
                        pe = psep.tile([128, 512], F32, name="pe", tag="pse", space="PSUM")
                        nc.tensor.matmul(pe[:], qu_lo[:, 128 * mt:128 * (mt + 1)],
                                         ku_lo[:, 512 * nf:512 * (nf + 1)], start=True, stop=False)
                        nc.tensor.matmul(pe[:], qu_hi[:, 128 * mt:128 * (mt + 1)],
                                         ku_hi[:, 512 * nf:512 * (nf + 1)], start=False, stop=True)
                        nc.scalar.copy(esb[:, 512 * nf:512 * (nf + 1)], pe[:])
                    mx = tinyp.tile([128, 8], F32, name="mx", tag="tiny8")
                    ix = idxp.tile([128, 8], U32, name="ix", tag="ix")
                    nc.vector.max(mx[:], esb[:])
                    nc.vector.max_index(ix[:], mx[:], esb[:])
                    nc.sync.dma_start(out=cv_d[b, 0, 128 * mt:128 * (mt + 1), 0:8], in_=ix[:])
                    nc.scalar.dma_start(out=cv_d[b, 0, 128 * mt:128 * (mt + 1), 8:10],
                                        in_=mx[:, 0:2].bitcast(U32))
                    if with_x2:
                        esn = esbp.tile([128, N], F32, name="esn", tag="esb")
                        nc.scalar.mul(esn[:], esb[:], -1.0)
                        mn = tinyp.tile([128, 8], F32, name="mn", tag="tiny8")
                        inx = idxp.tile([128, 8], U32, name="inx", tag="ix")
                        nc.vector.max(mn[:], esn[:])
                        nc.vector.max_index(inx[:], mn[:], esn[:])
                        nc.sync.dma_start(out=cv_d[b, 1, 128 * mt:128 * (mt + 1), 0:8], in_=inx[:])
                        nc.scalar.dma_start(out=cv_d[b, 1, 128 * mt:128 * (mt + 1), 8:10],
                                            in_=mn[:, 0:2].bitcast(U32))

    # wait-splitting post-pass (walrus in this container allows 1 sync-wait/inst)
    for f in nc.m.functions:
        for blk in f.blocks:
            newlist = []
            for i in blk.instructions:
                si = i.sync_info
                if si is not None and len(si.on_wait) > 1:
                    waits = list(si.on_wait)
                    keep = waits[-1:]
                    rest = waits[:-1]
                    for j, wchunk in enumerate(rest):
                        nop = mybir.InstNoOp(name=f"{i.name}-ws-{j}", ins=[], outs=[])
                        nop.engine = i.engine
                        nop.sync_info = mybir.SyncInfo(on_wait=[wchunk], on_update=[])
                        newlist.append(nop)
                    si.on_wait = keep
                newlist.append(i)
            blk.instructions[:] = newlist
    return nc


def _get_program(with_x2: bool):
    if with_x2 not in _CACHE:
        _CACHE[with_x2] = _build(with_x2)
    return _CACHE[with_x2]


_RUNNERS = {}
_ZSTASH = {}


def _get_runner(with_x2: bool):
    """Cached jitted SPMD runner taking FULL (unsharded) input arrays."""
    if with_x2 in _RUNNERS:
        return _RUNNERS[with_x2]
    import jax
    import concourse.mybir as mybir
    from concourse import bass2jax
    from jax.experimental.shard_map import shard_map
    from jax.sharding import Mesh, PartitionSpec, NamedSharding

    nc = _get_program(with_x2)
    bass2jax.install_neuronx_cc_hook()

    partition_name = nc.partition_id_tensor.name if nc.partition_id_tensor else None
    in_names, out_names, out_avals = [], [], []
    for alloc in nc.m.functions[0].allocations:
        if not isinstance(alloc, mybir.MemoryLocationSet):
            continue
        name = alloc.memorylocations[0].name
        if alloc.kind == "ExternalInput":
            if name != partition_name:
                in_names.append(name)
        elif alloc.kind == "ExternalOutput":
            out_names.append(name)
            out_avals.append(jax.core.ShapedArray(tuple(alloc.tensor_shape),
                                                  mybir.dt.np(alloc.dtype)))
    n_params = len(in_names)
    n_outs = len(out_avals)
    all_in_names = list(in_names) + list(out_names)
    if partition_name is not None:
        all_in_names.append(partition_name)

    def _body(*args):
        operands = list(args)
        if partition_name is not None:
            operands.append(bass2jax.partition_id_tensor())
        outs = bass2jax._bass_exec_p.bind(
            *operands,
            out_avals=tuple(out_avals),
            in_names=tuple(all_in_names),
            out_names=tuple(out_names),
            lowering_input_output_aliases=(),
            sim_require_finite=True,
            sim_require_nnan=True,
            nc=nc,
        )
        return tuple(outs)

    devices = jax.devices()[:N_CORES]
    mesh = Mesh(np.asarray(devices), ("core",))
    donate = tuple(range(n_params, n_params + n_outs))
    sharded = jax.jit(
        shard_map(_body, mesh=mesh,
                  in_specs=(PartitionSpec("core"),) * (n_params + n_outs),
                  out_specs=(PartitionSpec("core"),) * n_outs,
                  check_rep=False),
        donate_argnums=donate, keep_unused=True,
    )
    sharding = NamedSharding(mesh, PartitionSpec("core"))
    zero_shapes = [(N_CORES * a.shape[0], *a.shape[1:]) for a in out_avals]
    zero_dtypes = [a.dtype for a in out_avals]
    make_zeros = jax.jit(
        lambda: tuple(jax.numpy.zeros(s, d) for s, d in zip(zero_shapes, zero_dtypes)),
        out_shardings=(sharding,) * n_outs,
    )
    runner = (sharded, make_zeros, in_names, out_names, sharding, n_outs)
    _RUNNERS[with_x2] = runner
    return runner


if _torch is not None:
    def _cast_f16(dst, src):
        _torch.from_numpy(dst).copy_(_torch.from_numpy(src))
else:
    def _cast_f16(dst, src):
        dst[...] = src


def _unfold_f32(a):
    # [nb,H,W] -> [nb, N, 256] patch-major
    nb = a.shape[0]
    return np.ascontiguousarray(
        a.reshape(nb, NB, KP, NB, KP).transpose(0, 1, 3, 2, 4)).reshape(nb, N, PD)


def _refine(cand, val, q32, ku32, minimize):
    """Exact fp32 re-rank of device top-8 candidates where the fp16 margin is
    ambiguous.  cand [nb,N,8], val [nb,N,2], q32 [nb,H*W], ku32 [nb,N,256];
    returns [nb, N] selected patch index."""
    nb = cand.shape[0]
    sel_idx = cand[:, :, 0].astype(np.int64)
    amb = (val[:, :, 0] - val[:, :, 1]) < TH
    sgn = -1.0 if minimize else 1.0
    q6 = q32.reshape(nb, NB, KP, NB, KP)
    for b in range(nb):
        ms = np.nonzero(amb[b])[0]
        if ms.size == 0:
            continue
        cb = cand[b, ms].astype(np.int64)                 # [R, 8]
        kq = ku32[b][cb]                                  # [R, 8, 256]
        nr = np.sqrt(np.einsum('rcp,rcp->rc', kq, kq))
        kq = kq / np.maximum(nr, 1e-12)[:, :, None]
        qq = q6[b, ms >> 5, :, ms & 31, :].reshape(-1, PD)
        qq = qq / np.maximum(np.sqrt(np.einsum('rp,rp->r', qq, qq)), 1e-12)[:, None]
        sc = sgn * np.einsum('rcp,rp->rc', kq, qq)
        best = sc.max(1)
        tie = sc >= best[:, None]                         # exact-score winners
        big = np.where(tie, cb, np.int64(1 << 62))
        sel_idx[b, ms] = big.min(1)                       # first-occurrence tiebreak
    return sel_idx


def _gather_fold(x, idx, out):
    """out = fold(take(unfold(x), idx)) via one segment-level gather
    (a segment = the 16 contiguous floats of one patch row).
    x: [nb,C,H,W] f32 (contiguous), idx: [nb,N], out: [nb,C,H,W] contiguous."""
    nb = x.shape[0]
    hb = (idx >> 5).astype(np.int32).reshape(nb, NB, NB)  # [b, hb, wb]
    wb = (idx & 31).astype(np.int32).reshape(nb, NB, NB)
    bc = (np.arange(nb * C, dtype=np.int32) * H).reshape(nb, C)
    base = (hb[:, None, :, None, :] * np.int32(KP)
            + np.arange(KP, dtype=np.int32)[None, None, None, :, None]
            + bc[:, :, None, None, None])
    segidx = base * np.int32(NB) + wb[:, None, :, None, :]
    import os, time as _t
    dbg = os.environ.get("KV2_DEBUG")
    t0 = _t.time()
    if _torch is not None and os.environ.get("KV2_GATHER", "torch") == "torch":
        si = segidx.ravel().astype(np.int64)
        t1 = _t.time()
        _torch.index_select(_torch.from_numpy(x.reshape(-1, KP)), 0,
                            _torch.from_numpy(si),
                            out=_torch.from_numpy(out.reshape(-1, KP)))
        if dbg:
            print(f"    [gf] idx {t1-t0:.3f} isel {_t.time()-t1:.3f}", flush=True)
    else:
        xseg = x.view(np.dtype((np.void, KP * 4))).reshape(-1)
        oseg = out.view(np.dtype((np.void, KP * 4))).reshape(-1)
        np.take(xseg, segidx.ravel(), out=oseg)
        if dbg:
            print(f"    [gf] np.take total {_t.time()-t0:.3f}", flush=True)
    return out


def kernel(**inputs) -> np.ndarray:
    from concourse.bass_utils import run_bass_kernel_spmd  # noqa: F401 (axon redirects through bass2jax)
    import jax
    import os, time as _time
    _dbg = os.environ.get("KV2_DEBUG")
    _t0 = _time.time()
    _tick = (lambda tag: print(f"  [kv2] {tag} @ {_time.time()-_t0:.3f}s", flush=True)) if _dbg else (lambda tag: None)

    fe = np.asarray(inputs["feat_edit"], dtype=np.float32)
    fo = np.asarray(inputs["feat_ori"], dtype=np.float32)
    x1 = np.ascontiguousarray(np.asarray(inputs["x1"], dtype=np.float32))
    wq = np.asarray(inputs["wq"], dtype=np.float32).reshape(C)
    bq = np.float32(np.asarray(inputs["bq"], dtype=np.float32).reshape(()))
    wk = np.asarray(inputs["wk"], dtype=np.float32).reshape(C)
    bk = np.float32(np.asarray(inputs["bk"], dtype=np.float32).reshape(()))
    gamma2 = np.float32(np.asarray(inputs["gamma2"], dtype=np.float32).reshape(()))

    with_x2 = bool(gamma2 != 0.0)
    sharded, make_zeros, in_names, out_names, sharding, n_outs = _get_runner(with_x2)
    cv_pos = out_names.index("cv")

    zeros = _ZSTASH.pop(with_x2, None)
    if zeros is None:
        zeros = list(make_zeros())

    # conv/cast chunk j on CPU while chunk j-1 streams up (device_put is async)
    q32 = np.empty((B, H * W), np.float32)
    k32 = np.empty((B, H * W), np.float32)
    fer = fe.reshape(B, C, H * W)
    forr = fo.reshape(B, C, H * W)
    devs = {}
    for j in range(IPC):
        chunk = np.empty((N_CORES, 2, H, W), np.float16)
        for c in range(N_CORES):
            b = c * IPC + j
            np.dot(wq, fer[b], out=q32[b])
            q32[b] += bq
            _cast_f16(chunk[c, 0], q32[b].reshape(H, W))
            np.dot(wk, forr[b], out=k32[b])
            k32[b] += bk
            _cast_f16(chunk[c, 1], k32[b].reshape(H, W))
        devs[f"qk{j}"] = jax.device_put(chunk, sharding)
        _tick(f"chunk{j} conv+put")
    args = [devs[n] for n in in_names] + zeros
    out_arrs = sharded(*args)
    _tick("dispatch")
    _ZSTASH[with_x2] = list(make_zeros())   # async; ready by the next call

    # overlap the device round trip with the k unfold needed for refinement
    ku32 = _unfold_f32(k32.reshape(B, H, W))
    _tick("unfold")

    cv = np.asarray(out_arrs[cv_pos]).reshape(B, -1, N, 10)   # [B, NO, N, 10]
    _tick("cv fetched")
    cand = cv[..., 0:8]
    val = np.ascontiguousarray(cv[..., 8:10]).view(np.float32)

    out = np.empty((B, C, H, W), np.float32)
    am = _refine(cand[:, 0], val[:, 0], q32, ku32, minimize=False)
    _tick("refine")
    _gather_fold(x1, am, out)
    _tick("gather")
    if with_x2:
        an = _refine(cand[:, 1], val[:, 1], q32, ku32, minimize=True)
        x2 = np.ascontiguousarray(np.asarray(inputs["x2"], dtype=np.float32))
        o2 = np.empty((B, C, H, W), np.float32)
        _gather_fold(x2, an, o2)
        out += gamma2 * o2
    return out
